# revision 1
# baseline (speedup 1.0000x reference)
"""Trainium2 kernel for 4096x4096 single-channel 7x7 valid cross-correlation + bias.

Strategy (v2)
-------------
Conv decomposed into 7 banded-Toeplitz matmuls accumulated in PSUM:

    y[r, c] = sum_j sum_i W[i, j] * x[r+i, c+j]

Per strip of 128 input rows ([K=128 partitions, width] SBUF tile), kernel
column j contributes one TensorEngine matmul:
    lhsT = T_j [128, 128] with T_j[k, m] = W[k-m, j]   (stationary, banded)
    rhs  = X[:, j : j+512]                              (free-dim shift)
accumulating 122 valid output rows x 512 output cols in one PSUM bank.

Sharding: columns across 8 cores (512 output cols each + 6-col halo sliced
host-side).  34 row strips per core.

v2 performance structure (from the v1 neuron-profile trace):
- j-outer over groups of 8 strips: consecutive matmuls share lhsT, which
  lets the PE's LDWEIGHTS pipeline behind the running matmul (v1's
  per-strip weight switching serialized ~100ns/MM of LDW into the chain).
- Input strips host-packed in pairs -> [17, 128, 2*518] so each load is
  one DMA with 2072-byte per-partition descriptors (2x v1).
- Output drained to bf16 (error budget allows it; rel err ~4e-3 vs 2e-3)
  and pair-packed -> [17, 122, 2*512]: halves store bytes, 2048B descrs.
- PSUM drains alternate ScalarE/VectorE so bank recycling keeps pace with
  the PE at group boundaries.
"""

import os

import numpy as np
import ml_dtypes

import concourse.bass as bass
import concourse.bacc as bacc_mod
import concourse.mybir as mybir
import concourse.tile as tile
from concourse.bass_utils import run_bass_kernel_spmd

H = 4096          # input rows
W = 4096          # input cols
KH = 7            # kernel rows
KW = 7            # kernel cols
OH = H - KH + 1   # 4090 output rows
OW = W - KW + 1   # 4090 output cols
NCORES = 8
CW = 512          # output cols per core
SW = CW + KW - 1  # 518 input cols per shard
STRIP = 122       # output rows per strip (128 input rows -> 122 outputs)
N_STRIPS = -(-OH // STRIP)   # 34
N_PAIRS = -(-N_STRIPS // 2)  # 17
GROUP = 8                    # strips per j-outer group (= PSUM banks)

_BF16 = ml_dtypes.bfloat16


def _strip_mk(s: int) -> tuple[int, int]:
    """(valid output rows, input rows) of strip s."""
    m = min(STRIP, OH - s * STRIP)
    return m, m + KH - 1


def _build_program(bias_val: float) -> bass.Bass:
    nc = bacc_mod.Bacc("TRN2", target_bir_lowering=False)

    x_d = nc.dram_tensor("xs", [N_PAIRS, 128, 2 * SW], mybir.dt.bfloat16,
                         kind="ExternalInput")
    t_d = nc.dram_tensor("tmat", [128, KW * 128], mybir.dt.bfloat16,
                         kind="ExternalInput")
    y_d = nc.dram_tensor("y", [N_PAIRS, STRIP, 2 * CW], mybir.dt.bfloat16,
                         kind="ExternalOutput")

    with tile.TileContext(nc) as tc:
        with (
            tc.tile_pool(name="const", bufs=1) as constp,
            tc.tile_pool(name="xg", bufs=N_PAIRS) as xgp,
            tc.tile_pool(name="yg", bufs=N_PAIRS) as ygp,
            tc.tile_pool(name="ps", bufs=GROUP, space="PSUM") as psp,
        ):
            t_sb = constp.tile([128, KW * 128], mybir.dt.bfloat16)
            nc.sync.dma_start(t_sb[:, :], t_d[:, :])

            xg_tiles = []
            for g in range(N_PAIRS):
                xg = xgp.tile([128, 2 * SW], mybir.dt.bfloat16, name="xg", tag="xg")
                nc.sync.dma_start(xg[:, :], x_d[g, :, :])
                xg_tiles.append(xg)

            for g0 in range(0, N_STRIPS, GROUP):
                strips = list(range(g0, min(g0 + GROUP, N_STRIPS)))

                ps_tiles = {}
                for s in strips:
                    ps_tiles[s] = psp.tile([128, CW], mybir.dt.float32, name="ps", tag="ps")

                for j in range(KW):
                    for s in strips:
                        m, k = _strip_mk(s)
                        mw = 128 if m == STRIP else m
                        xg = xg_tiles[s // 2]
                        off = (s % 2) * SW
                        nc.tensor.matmul(
                            ps_tiles[s][:mw, :],
                            t_sb[:k, j * 128:j * 128 + mw],
                            xg[:k, off + j:off + j + CW],
                            start=(j == 0),
                            stop=(j == KW - 1),
                        )

                for s in strips:
                    m, _ = _strip_mk(s)
                    g, h = s // 2, s % 2
                    if h == 0:
                        yg = ygp.tile([STRIP, 2 * CW], mybir.dt.bfloat16,
                                      name="yg", tag="yg")
                        yg_tiles = getattr(tc, "_yg_tiles", {})
                        yg_tiles[g] = yg
                        tc._yg_tiles = yg_tiles
                    else:
                        yg = tc._yg_tiles[g]
                    dst = yg[:m, h * CW:(h + 1) * CW]
                    src = ps_tiles[s][:m, :]
                    if s % 2 == 0:
                        nc.scalar.activation(
                            dst, src, mybir.ActivationFunctionType.Copy,
                            bias=float(bias_val),
                        )
                    else:
                        nc.vector.tensor_scalar_add(dst, src, float(bias_val))
                    if h == 1 or s == N_STRIPS - 1:
                        nc.gpsimd.dma_start(y_d[g, :, :], yg[:, :])

    nc.compile()
    nc.finalize()
    return nc


def _toeplitz(weight: np.ndarray) -> np.ndarray:
    """[128, 7*128] bf16; block j holds T_j[k, m] = W[k-m, j] (band 0<=k-m<7)."""
    t = np.zeros((128, KW * 128), np.float32)
    for j in range(KW):
        for i in range(KH):
            mm = np.arange(0, 128 - i)
            t[mm + i, j * 128 + mm] = weight[i, j]
    return t.astype(_BF16)


def _pack_shard(x_bf: np.ndarray, c0: int) -> np.ndarray:
    """[17, 128, 2*518] bf16: pair 2 strips per partition line."""
    valid = min(SW, W - c0)
    xs = np.zeros((H + 2 * STRIP, SW), _BF16)  # row padding for edge strips
    xs[:H, :valid] = x_bf[:, c0:c0 + valid]
    packed = np.zeros((N_PAIRS, 128, 2 * SW), _BF16)
    for g in range(N_PAIRS):
        packed[g, :, :SW] = xs[2 * g * STRIP: 2 * g * STRIP + 128]
        packed[g, :, SW:] = xs[(2 * g + 1) * STRIP: (2 * g + 1) * STRIP + 128]
    return packed


def _unpack_out(y_packed: np.ndarray) -> np.ndarray:
    """[17, 122, 1024] bf16 -> [4090, 512] f32."""
    out = np.empty((OH, CW), np.float32)
    for s in range(N_STRIPS):
        m, _ = _strip_mk(s)
        g, h = s // 2, s % 2
        out[s * STRIP: s * STRIP + m, :] = \
            y_packed[g, :m, h * CW:(h + 1) * CW].astype(np.float32)
    return out


def kernel(x: np.ndarray, weight: np.ndarray, bias: np.ndarray) -> np.ndarray:
    x = np.asarray(x, dtype=np.float32)
    weight = np.asarray(weight, dtype=np.float32)
    bias = np.asarray(bias, dtype=np.float32)

    tmat = _toeplitz(weight)
    x_bf = x.astype(_BF16)

    in_maps = []
    for c in range(NCORES):
        in_maps.append({"xs": _pack_shard(x_bf, CW * c), "tmat": tmat})

    nc = _build_program(float(bias[0]))

    trace = bool(int(os.environ.get("CONV_KERNEL_TRACE", "0")))
    res = run_bass_kernel_spmd(nc, in_maps, core_ids=list(range(NCORES)),
                               trace=trace)
    if trace:
        kernel.last_exec_time_ns = res.exec_time_ns

    cols = []
    for c in range(NCORES):
        valid_out = min(CW, OW - CW * c)
        cols.append(_unpack_out(np.asarray(res.results[c]["y"]))[:, :valid_out])
    return np.concatenate(cols, axis=1).astype(np.float32)



# revision 5
# speedup vs baseline: 1.1251x; 1.1251x over previous
"""Trainium2 kernel for 4096x4096 single-channel 7x7 valid cross-correlation + bias.

Strategy (v3): 32x32 PE-array tiling
------------------------------------
Conv = 7 banded-Toeplitz matmuls accumulated in PSUM (one per kernel column j):
    y[r, c] = sum_j sum_i W[i, j] * x[r+i, c+j]
With a 128x128 matmul the band (7 diagonals) uses only ~5% of the PE array.
The TRN2 PE is physically 16 independent 32x32 sub-arrays; `tile_position`
addresses them and concurrent tiles stream independently (measured 10.6x for
16-tile K=M=32 N=512 packs).  So:

- Strips of 32 input rows -> 26 output rows, K=32 contraction, M=32 outputs.
- 16 strips per round run on the 16 sub-arrays concurrently; each strip is 7
  PSUM-accumulated matmuls (N=512 output cols), lhsT = 32x32 Toeplitz slice.
- Strip 16R+4a+c' streams from SBUF partitions 32c' (row-group c') and
  accumulates into PSUM bank c', partitions 32a (col-group a).
  4 banks per round, double-buffered across the 8 banks.
- Drains: bank -> bf16 + bias into a [128, 4*512] staging tile, alternating
  ScalarE/VectorE; one output DMA per (round, a-quarter) of the 26 valid rows.

Sharding: output columns across 8 cores (512 cols each, 518 input cols with
the 6-col halo sliced host-side).  Per core: 160 strip slots = 10 rounds,
40 input tiles [128, 518] (4 strips each, replicated boundary rows),
pair-packed into 20 DMAs with 2072-byte partition lines.
"""

import os

import numpy as np
import ml_dtypes

import concourse.bass as bass
import concourse.bacc as bacc_mod
import concourse.mybir as mybir
import concourse.tile as tile
from concourse.bass_utils import run_bass_kernel_spmd

H = 4096          # input rows
W = 4096          # input cols
KH = 7            # kernel rows
KW = 7            # kernel cols
OH = H - KH + 1   # 4090 output rows
OW = W - KW + 1   # 4090 output cols
NCORES = 8
CW = 512          # output cols per core
SW = CW + KW - 1  # 518 input cols per shard

TS = 32           # input rows per strip (PE tile contraction)
SOUT = TS - KH + 1  # 26 output rows per strip
NROUNDS = 10      # 16 strips per round
NSTRIPS = 16 * NROUNDS          # 160 strip slots (158 carry real rows)
NTILES_IN = 4 * NROUNDS         # 40 input tiles, 4 strips each
NDMA_IN = NTILES_IN // 2        # pair-packed input DMAs
ROWS_PER_TILE = 4 * SOUT        # 104-row advance per input tile
PAD_ROWS = ROWS_PER_TILE * (NTILES_IN - 1) + 3 * SOUT + TS  # 4166

_BF16 = ml_dtypes.bfloat16


def _build_program(bias_val: float) -> bass.Bass:
    nc = bacc_mod.Bacc("TRN2", target_bir_lowering=False)

    x_d = nc.dram_tensor("xs", [NDMA_IN, 128, 2 * SW], mybir.dt.bfloat16,
                         kind="ExternalInput")
    w_d = nc.dram_tensor("tmat", [128, KW * TS], mybir.dt.bfloat16,
                         kind="ExternalInput")
    # y[R, a, q, cp, w]: strip 16R+4a+cp, output row q, col w — this
    # iteration order matches the SBUF source [26 partitions, 4*512 cols]
    # so each (round, a) store is a single straight DMA.
    y_d = nc.dram_tensor("y", [NROUNDS, 4, SOUT, 4, CW], mybir.dt.bfloat16,
                         kind="ExternalOutput")

    with tile.TileContext(nc) as tc:
        with (
            tc.tile_pool(name="const", bufs=1) as constp,
            tc.tile_pool(name="xg", bufs=NDMA_IN) as xgp,
            tc.tile_pool(name="yg", bufs=NROUNDS) as ygp,
            tc.tile_pool(name="ps", bufs=8, space="PSUM") as psp,
        ):
            w_sb = constp.tile([128, KW * TS], mybir.dt.bfloat16)
            nc.sync.dma_start(w_sb[:, :], w_d[:, :])

            xg_tiles = []
            for g in range(NDMA_IN):
                xg = xgp.tile([128, 2 * SW], mybir.dt.bfloat16,
                              name="xg", tag="xg")
                nc.sync.dma_start(xg[:, :], x_d[g, :, :])
                xg_tiles.append(xg)

            for R in range(NROUNDS):
                ps_tiles = [psp.tile([128, CW], mybir.dt.float32,
                                     name="ps", tag="ps")
                            for _ in range(4)]

                for j in range(KW):
                    for a in range(4):
                        g = 4 * R + a
                        xg = xg_tiles[g // 2]
                        off = (g % 2) * SW + j
                        for cp in range(4):
                            nc.tensor.matmul(
                                ps_tiles[cp][32 * a:32 * a + 32, :],
                                w_sb[32 * cp:32 * cp + 32,
                                     TS * j:TS * j + TS],
                                xg[32 * cp:32 * cp + 32, off:off + CW],
                                start=(j == 0),
                                stop=(j == KW - 1),
                                tile_position=(32 * cp, 32 * a),
                            )

                yt = ygp.tile([128, 4 * CW], mybir.dt.bfloat16,
                              name="yg", tag="yg")
                for cp in range(4):
                    dst = yt[:, cp * CW:(cp + 1) * CW]
                    src = ps_tiles[cp][:, :]
                    if cp % 2 == 0:
                        nc.scalar.activation(
                            dst, src, mybir.ActivationFunctionType.Copy,
                            bias=float(bias_val),
                        )
                    else:
                        nc.vector.tensor_scalar_add(dst, src, float(bias_val))

                # 4 DMAs per round: the 26 valid rows of each col-group a
                # (strips 16R + 4a + cp for cp = 0..3).
                for a in range(4):
                    nc.gpsimd.dma_start(
                        y_d[R, a, :, :, :],
                        yt[32 * a:32 * a + SOUT, :],
                    )

    nc.compile()
    nc.finalize()
    return nc


def _toeplitz(weight: np.ndarray) -> np.ndarray:
    """[128, 7*32] bf16; block j holds T_j[k, m] = W[k-m, j] (band 0<=k-m<7),
    replicated across the 4 partition quadrants (one per PE row-group)."""
    t = np.zeros((TS, KW * TS), np.float32)
    for j in range(KW):
        for i in range(KH):
            mm = np.arange(0, TS - i)
            t[mm + i, j * TS + mm] = weight[i, j]
    return np.tile(t, (4, 1)).astype(_BF16)


def _pack_shard(x_bf: np.ndarray, c0: int) -> np.ndarray:
    """[20, 128, 2*518] bf16: input tile g holds strips 4g..4g+3 at partition
    quadrants (partition 32c+q = row 104g + 26c + q); tiles pair-packed."""
    valid = min(SW, W - c0)
    xs = np.zeros((PAD_ROWS, SW), _BF16)
    xs[:H, :valid] = x_bf[:, c0:c0 + valid]
    p = np.arange(128)
    rows = (ROWS_PER_TILE * np.arange(NTILES_IN)[:, None]
            + SOUT * (p // 32) + (p % 32))          # [40, 128]
    packed = xs[rows]                               # [40, 128, 518]
    return np.ascontiguousarray(
        packed.reshape(NDMA_IN, 2, 128, SW).transpose(0, 2, 1, 3)
        .reshape(NDMA_IN, 128, 2 * SW))


def _unpack_out(y_packed: np.ndarray) -> np.ndarray:
    """[10, 4, 26, 4, 512] bf16 -> [4090, 512] f32 (strip 16R+4a+cp row q)."""
    y = y_packed.reshape(NROUNDS, 4, SOUT, 4, CW).transpose(0, 1, 3, 2, 4)
    return y.reshape(NSTRIPS * SOUT, CW)[:OH].astype(np.float32)


def kernel(x: np.ndarray, weight: np.ndarray, bias: np.ndarray) -> np.ndarray:
    x = np.asarray(x, dtype=np.float32)
    weight = np.asarray(weight, dtype=np.float32)
    bias = np.asarray(bias, dtype=np.float32)

    tmat = _toeplitz(weight)
    x_bf = x.astype(_BF16)

    in_maps = []
    for c in range(NCORES):
        in_maps.append({"xs": _pack_shard(x_bf, CW * c), "tmat": tmat})

    nc = _build_program(float(bias[0]))

    trace = bool(int(os.environ.get("CONV_KERNEL_TRACE", "0")))
    res = run_bass_kernel_spmd(nc, in_maps, core_ids=list(range(NCORES)),
                               trace=trace)
    if trace:
        kernel.last_exec_time_ns = res.exec_time_ns

    cols = []
    for c in range(NCORES):
        valid_out = min(CW, OW - CW * c)
        cols.append(_unpack_out(np.asarray(res.results[c]["y"]))[:, :valid_out])
    return np.concatenate(cols, axis=1).astype(np.float32)
